# revision 2
# baseline (speedup 1.0000x reference)
"""Distributed Trainium2 Bass kernel for a single attention head.

Problem (hardcoded): q,k,v [4, 4096, 1024] f32, Wq/Wk/Wv [1024, 64] f32,
attn_mask [4096, 4096] bool (True = keep).  out[b] = softmax(mask(q Wq (k Wk)^T) / 8) (v Wv).

Sharding: 8 cores; core c -> batch c//2, and query chunks {2s + c%2 : s in 0..3}
(512 rows each, interleaved so the causal-mask work per program slot is uniform
across cores -- all cores execute one SPMD program).

v2: all matmul operands are bf16 (host casts), halving the dominant HBM
traffic.  The additive mask is applied from a small resident SBUF pattern
table ([NJ,128,512], NJ=8 for the causal mask): for the tril mask the
pattern at (slot s, k-tile 8s+j) depends only on (j, core parity), so per
slot only the window of partially-masked tiles needs adds and the tile
content is shared across slots.  v-projection computes vh[k,h] directly
(lhsT = vT tile, rhs = Wv) so no PE transposes or copies are needed.
PSUM->SBUF movement: exp on the Act engine, proj copies + mask adds on DVE.
"""

import os
import sys

sys.path.insert(0, "/opt/trn_rl_repo")

import numpy as np
import ml_dtypes

import concourse.bass as bass
import concourse.mybir as mybir
import concourse.tile as tile
from concourse import bacc
from concourse.bass_utils import run_bass_kernel_spmd
from concourse.masks import make_identity

F32 = mybir.dt.float32
BF16 = mybir.dt.bfloat16
BF = ml_dtypes.bfloat16

N_CORES = 8
B, T, D, H = 4, 4096, 1024, 64
P = 128                      # partitions
QC = 512                     # query chunk width
N_CHUNKS = T // QC           # 8 global query chunks
N_SLOTS = N_CHUNKS // 2      # 4 chunks per core
KT = T // P                  # 32 k-tiles of 128 rows
D_TILES = D // P             # 8
XCW = 1024                   # x-chunk width for streaming projections
TQ = N_SLOTS * QC            # 2048 local query rows per core
NEG = -1.0e30                # additive mask value for dropped positions

LAST_RESULT = None           # test harness reads exec_time_ns from here
_CACHE = {}


def _mask_schedule(mask):
    """Derive the compile-time schedule from the actual mask.

    Returns (extents, window, resident):
      extents[s]: #k-tiles processed for slot s (valid for both parities)
      window[s]:  tuple of k-tiles t < extents[s] needing an additive mask
      resident:   True if for each parity the j-th window tile's content is
                  identical across slots (then an [NJ,128,512] SBUF pattern
                  table indexed by j suffices; NJ = max window length).
    """
    m = mask.reshape(N_CHUNKS, QC, KT, P)
    blk_any = m.any(axis=(1, 3))   # [chunk, ktile]
    blk_all = m.all(axis=(1, 3))
    extents = []
    window = []
    for s in range(N_SLOTS):
        js = (2 * s, 2 * s + 1)
        ext = 1
        for j in js:
            nz = np.nonzero(blk_any[j])[0]
            if len(nz):
                ext = max(ext, int(nz[-1]) + 1)
        extents.append(ext)
        window.append(tuple(
            t for t in range(ext) if (~blk_all[js, t]).any()))
    nj = max((len(w) for w in window), default=0)
    resident = nj <= 16
    if resident:
        for par in range(2):
            for j in range(nj):
                blocks = [
                    m[2 * s + par, :, window[s][j], :]
                    for s in range(N_SLOTS) if j < len(window[s])
                ]
                if any(not np.array_equal(blocks[0], bb) for bb in blocks[1:]):
                    resident = False
    return tuple(extents), tuple(window), resident


def _build(extents, window, resident):
    nj = max((len(w) for w in window), default=0)
    n_mask = max(1, nj if resident else sum(len(w) for w in window))
    nc = bacc.Bacc("TRN2", target_bir_lowering=False, debug=False,
                   num_devices=N_CORES)
    qT = nc.dram_tensor("qT", [D, TQ], BF16, kind="ExternalInput")
    kT = nc.dram_tensor("kT", [D, T], BF16, kind="ExternalInput")
    vT = nc.dram_tensor("vT", [D, T], BF16, kind="ExternalInput")
    w = nc.dram_tensor("w", [D, 3 * H], BF16, kind="ExternalInput")
    maskp = nc.dram_tensor("maskp", [n_mask, P, QC], F32, kind="ExternalInput")
    out = nc.dram_tensor("out", [TQ, H], F32, kind="ExternalOutput")

    Exp = mybir.ActivationFunctionType.Exp
    kt_lim = max(extents)
    n_kv_chunks = (kt_lim * P + XCW - 1) // XCW
    # j index per (s, t) for window tiles
    jmap = {}
    mask_idx = 0
    for s in range(N_SLOTS):
        for j, t in enumerate(window[s]):
            jmap[(s, t)] = j
    # emission order of streamed mask tiles (t-outer like the loop below)
    stream_order = {}
    for t in range(kt_lim):
        for s in range(N_SLOTS):
            if t < extents[s] and (s, t) in jmap:
                stream_order[(s, t)] = len(stream_order)

    with tile.TileContext(nc) as tc:
        with (
            tc.tile_pool(name="const", bufs=1) as cpool,
            tc.tile_pool(name="qkh", bufs=1) as qkhpool,
            tc.tile_pool(name="vh", bufs=1) as vhpool,
            tc.tile_pool(name="oacc", bufs=1, space="PSUM") as opool,
        ):
            w_sb = cpool.tile([P, D_TILES, 3 * H], BF16)
            nc.sync.dma_start(
                out=w_sb[:], in_=w.ap().rearrange("(dt p) n -> p dt n", p=P))
            ident = cpool.tile([P, P], F32)
            make_identity(nc, ident[:])
            if resident and nj:
                maskp_sb = cpool.tile([P, nj, QC], F32)
                nc.sync.dma_start(
                    out=maskp_sb[:],
                    in_=maskp.ap().rearrange("j p q -> p j q"))

            qhT = qkhpool.tile([H, TQ], BF16, tag="qhT")
            khT = qkhpool.tile([H, T], BF16, tag="khT")
            vh1 = vhpool.tile([P, KT, H + 1], BF16)
            nc.vector.memset(vh1[:, :, H:H + 1], 1.0)

            oaccs = [opool.tile([H + 1, QC], F32, tag=f"oacc{s}",
                                name=f"oacc{s}")
                     for s in range(N_SLOTS)]
            started = [False] * N_SLOTS

            with (
                tc.tile_pool(name="xs", bufs=2) as xpool,
                tc.tile_pool(name="pps", bufs=2, space="PSUM") as pppool,
                tc.tile_pool(name="sps", bufs=2, space="PSUM") as spool,
                tc.tile_pool(name="pt", bufs=3) as ppool,
                tc.tile_pool(name="mt", bufs=3) as mpool,
                tc.tile_pool(name="osb", bufs=2) as osbpool,
                tc.tile_pool(name="rec", bufs=2) as recpool,
                tc.tile_pool(name="ob", bufs=2) as obpool,
            ):
                def epilogue(s):
                    osb = osbpool.tile([H + 1, QC], F32, tag="osb")
                    nc.vector.tensor_scalar_mul(osb[:], oaccs[s][:], 1.0)
                    for jj in range(QC // P):
                        ot = pppool.tile([P, H + 1], F32, tag="pp")
                        nc.tensor.transpose(
                            ot[:], osb[:, jj * P:(jj + 1) * P],
                            ident[0:H + 1, 0:H + 1])
                        rec = recpool.tile([P, 1], F32, tag="rec")
                        nc.vector.reciprocal(rec[:], ot[:, H:H + 1])
                        ob = obpool.tile([P, H], F32, tag="ob")
                        nc.vector.tensor_scalar_mul(ob[:], ot[:, 0:H], rec[:])
                        r0 = s * QC + jj * P
                        nc.sync.dma_start(out=out[r0:r0 + P, :], in_=ob[:])

                # ---- phase 0: project q -> qhT [64, TQ] ----
                for ch in range(TQ // XCW):
                    xt = xpool.tile([P, D_TILES, XCW], BF16, tag="x")
                    nc.sync.dma_start(
                        out=xt[:],
                        in_=qT[:, ch * XCW:(ch + 1) * XCW].rearrange(
                            "(dt p) t -> p dt t", p=P))
                    for n in range(XCW // QC):
                        ps = pppool.tile([H, QC], F32, tag="pp")
                        for dt_ in range(D_TILES):
                            nc.tensor.matmul(
                                ps[:],
                                lhsT=w_sb[:, dt_, 0:H],
                                rhs=xt[:, dt_, n * QC:(n + 1) * QC],
                                start=(dt_ == 0), stop=(dt_ == D_TILES - 1))
                        col = ch * XCW + n * QC
                        nc.vector.tensor_scalar_mul(
                            qhT[:, col:col + QC], ps[:], 1.0)

                # ---- phase 1: stream k/v chunks; project; attention tiles ----
                for ch in range(n_kv_chunks):
                    # k projection -> khT columns
                    ktx = xpool.tile([P, D_TILES, XCW], BF16, tag="x")
                    nc.sync.dma_start(
                        out=ktx[:],
                        in_=kT[:, ch * XCW:(ch + 1) * XCW].rearrange(
                            "(dt p) t -> p dt t", p=P))
                    for n in range(XCW // QC):
                        col = ch * XCW + n * QC
                        ps = pppool.tile([H, QC], F32, tag="pp")
                        for dt_ in range(D_TILES):
                            nc.tensor.matmul(
                                ps[:],
                                lhsT=w_sb[:, dt_, H:2 * H],
                                rhs=ktx[:, dt_, n * QC:(n + 1) * QC],
                                start=(dt_ == 0), stop=(dt_ == D_TILES - 1))
                        nc.vector.tensor_scalar_mul(
                            khT[:, col:col + QC], ps[:], 1.0)
                    # v projection -> vh1[k, h] directly (no transpose)
                    vtx = xpool.tile([P, D_TILES, XCW], BF16, tag="x")
                    nc.sync.dma_start(
                        out=vtx[:],
                        in_=vT[:, ch * XCW:(ch + 1) * XCW].rearrange(
                            "(dt p) t -> p dt t", p=P))
                    for kt_ in range(XCW // P):
                        t_glob = ch * (XCW // P) + kt_
                        vp = pppool.tile([P, H], F32, tag="pp")
                        for dt_ in range(D_TILES):
                            nc.tensor.matmul(
                                vp[:],
                                lhsT=vtx[:, dt_, kt_ * P:(kt_ + 1) * P],
                                rhs=w_sb[:, dt_, 2 * H:3 * H],
                                start=(dt_ == 0), stop=(dt_ == D_TILES - 1))
                        nc.vector.tensor_scalar_mul(
                            vh1[:, t_glob, 0:H], vp[:], 1.0)

                    # attention tiles for the k-tiles this chunk covers
                    for t in range(ch * (XCW // P), (ch + 1) * (XCW // P)):
                        if t >= kt_lim:
                            continue
                        live = [s for s in range(N_SLOTS) if t < extents[s]]
                        sts = {}
                        for s in live:
                            sp = spool.tile([P, QC], F32, tag="S")
                            nc.tensor.matmul(
                                sp[:],
                                lhsT=khT[:, t * P:(t + 1) * P],
                                rhs=qhT[:, s * QC:(s + 1) * QC],
                                start=True, stop=True)
                            if (s, t) in jmap:
                                if resident:
                                    nc.vector.tensor_add(
                                        sp[:], sp[:],
                                        maskp_sb[:, jmap[(s, t)], :])
                                else:
                                    m = mpool.tile([P, QC], F32, tag="m")
                                    nc.sync.dma_start(
                                        out=m[:],
                                        in_=maskp[stream_order[(s, t)]])
                                    nc.vector.tensor_add(sp[:], sp[:], m[:])
                            sts[s] = sp
                        for s in live:
                            p = ppool.tile([P, QC], BF16, tag="P")
                            nc.scalar.activation(
                                out=p[:], in_=sts[s][:], func=Exp, scale=0.125)
                            nc.tensor.matmul(
                                oaccs[s][:],
                                lhsT=vh1[:, t, :],
                                rhs=p[:],
                                start=not started[s],
                                stop=(t == extents[s] - 1))
                            started[s] = True
                        for s in live:
                            if t == extents[s] - 1:
                                epilogue(s)

    nc.compile()
    return nc


def _get_nc(extents, window, resident):
    key = (extents, window, resident)
    if key not in _CACHE:
        _CACHE[key] = _build(extents, window, resident)
    return _CACHE[key]


def _pack_w(Wq, Wk, Wv):
    return np.concatenate(
        [np.asarray(Wq), np.asarray(Wk), np.asarray(Wv)],
        axis=1).astype(BF)


def _make_in_maps(q, k, v, wcat, mask, extents, window, resident):
    nj = max((len(w) for w in window), default=0)
    kTb = [np.ascontiguousarray(k[b].T.astype(BF)) for b in range(B)]
    vTb = [np.ascontiguousarray(v[b].T.astype(BF)) for b in range(B)]
    qTb = [np.ascontiguousarray(q[b].T.astype(BF)) for b in range(B)]
    mm = mask.reshape(N_CHUNKS, QC, KT, P)

    def add_tile(g, t):
        # [128 k, 512 q] additive tile for (chunk g, k-tile t)
        return np.where(mm[g, :, t, :].T, np.float32(0.0), np.float32(NEG))

    in_maps = []
    for c in range(N_CORES):
        b, par = divmod(c, 2)
        chunks = [2 * s + par for s in range(N_SLOTS)]
        qT_core = np.ascontiguousarray(np.concatenate(
            [qTb[b][:, g * QC:(g + 1) * QC] for g in chunks], axis=1))
        if resident:
            if nj:
                tiles = []
                for j in range(nj):
                    s = next(s for s in range(N_SLOTS) if j < len(window[s]))
                    tiles.append(add_tile(chunks[s], window[s][j]))
                mp = np.stack(tiles).astype(np.float32)
            else:
                mp = np.zeros((1, P, QC), np.float32)
        else:
            order = sorted(
                ((s, t) for s in range(N_SLOTS) for t in window[s]),
                key=lambda st: (st[1], st[0]))
            mp = np.stack([add_tile(chunks[s], t) for (s, t) in order]
                          ).astype(np.float32)
        in_maps.append({
            "qT": qT_core, "kT": kTb[b], "vT": vTb[b],
            "w": wcat, "maskp": mp,
        })
    return in_maps


def _gather_out(results):
    outp = np.empty((B, T, H), np.float32)
    for c in range(N_CORES):
        b, par = divmod(c, 2)
        oc = results[c]["out"]
        for s in range(N_SLOTS):
            g = 2 * s + par
            outp[b, g * QC:(g + 1) * QC, :] = oc[s * QC:(s + 1) * QC, :]
    return outp


def kernel(q, k, v, Wq, Wk, Wv, attn_mask):
    global LAST_RESULT
    q = np.asarray(q, dtype=np.float32)
    k = np.asarray(k, dtype=np.float32)
    v = np.asarray(v, dtype=np.float32)
    mask = np.asarray(attn_mask).astype(bool)
    wcat = _pack_w(Wq, Wk, Wv)

    extents, window, resident = _mask_schedule(mask)
    nc = _get_nc(extents, window, resident)
    in_maps = _make_in_maps(q, k, v, wcat, mask, extents, window, resident)

    res = run_bass_kernel_spmd(
        nc, in_maps, core_ids=list(range(N_CORES)),
        trace=bool(os.environ.get("KBENCH_TRACE")))
    LAST_RESULT = res
    return _gather_out(res.results)


# revision 6
# speedup vs baseline: 1.0618x; 1.0618x over previous
"""Distributed Trainium2 Bass kernel for a single attention head.

Problem (hardcoded): q,k,v [4, 4096, 1024] f32, Wq/Wk/Wv [1024, 64] f32,
attn_mask [4096, 4096] bool (True = keep).  out[b] = softmax(mask(q Wq (k Wk)^T) / 8) (v Wv).

Sharding: 8 cores; core c -> batch c//2, and query chunks {2s + c%2 : s in 0..3}
(512 rows each, interleaved so the causal-mask work per program slot is uniform
across cores -- all cores execute one SPMD program).

v3: bf16 operands everywhere (host casts; halves HBM traffic).  The causal
mask is applied ON the tensor engine: for a k-suffix mask the additive
term NEG*(k >= k0(q)) equals Atri^T @ B with Atri[i,k] = NEG*(i<=k)
(constant) and B[i,q] one-hot at i=k0(q) (host data, [NJ,128,512] bf16
resident in SBUF) -- a second matmul accumulated into the score PSUM, so
no vector-engine add sits on the score->exp->PV chain.  Score PSUMs for
slot pairs share a [128,1024] tile so one exp instruction covers two
slots.  PV matmuls run one k-tile behind the scores (software pipeline)
so the PE never waits on the activation engine.  v-projection produces
[h,cols] wide matmuls, then PE-transposes into vh1[k,h].
"""

import os
import sys

sys.path.insert(0, "/opt/trn_rl_repo")

import numpy as np
import ml_dtypes

import concourse.bass as bass
import concourse.mybir as mybir
import concourse.tile as tile
from concourse import bacc
from concourse.bass_utils import run_bass_kernel_spmd
from concourse.masks import make_identity

F32 = mybir.dt.float32
BF16 = mybir.dt.bfloat16
BF = ml_dtypes.bfloat16

N_CORES = 8
B, T, D, H = 4, 4096, 1024, 64
P = 128                      # partitions
QC = 512                     # query chunk width
N_CHUNKS = T // QC           # 8 global query chunks
N_SLOTS = N_CHUNKS // 2      # 4 chunks per core
KT = T // P                  # 32 k-tiles of 128 rows
D_TILES = D // P             # 8
XCW = 2048                   # x-chunk width for streaming projections
TQ = N_SLOTS * QC            # 2048 local query rows per core
NEG = -1.0e30                # additive mask value for dropped positions

LAST_RESULT = None           # test harness reads exec_time_ns from here
_CACHE = {}


def _mask_schedule(mask):
    """Derive the compile-time schedule from the actual mask.

    Returns (extents, window, mode):
      extents[s]: #k-tiles processed for slot s (valid for both parities)
      window[s]:  tuple of k-tiles t < extents[s] needing masking
      mode: 'mm' if every window block is a k-suffix drop (mask applied as
            a matmul), 'add' if blocks are consistent across slots but not
            suffix (resident additive tiles + DVE add), else 'stream'.
    """
    m = mask.reshape(N_CHUNKS, QC, KT, P)
    blk_any = m.any(axis=(1, 3))   # [chunk, ktile]
    blk_all = m.all(axis=(1, 3))
    extents = []
    window = []
    for s in range(N_SLOTS):
        js = (2 * s, 2 * s + 1)
        ext = 1
        for j in js:
            nz = np.nonzero(blk_any[j])[0]
            if len(nz):
                ext = max(ext, int(nz[-1]) + 1)
        extents.append(ext)
        window.append(tuple(
            t for t in range(ext) if (~blk_all[js, t]).any()))
    nj = max((len(w) for w in window), default=0)
    consistent = nj <= 16
    suffix = True
    if consistent:
        for par in range(2):
            for j in range(nj):
                blocks = [
                    m[2 * s + par, :, window[s][j], :]
                    for s in range(N_SLOTS) if j < len(window[s])
                ]
                if any(not np.array_equal(blocks[0], bb) for bb in blocks[1:]):
                    consistent = False
                drop = ~blocks[0]            # [qr, kc]
                if not (drop[:, :-1] <= drop[:, 1:]).all():
                    suffix = False
    mode = ('mm' if consistent and suffix else
            'add' if consistent else 'stream')
    return tuple(extents), tuple(window), mode


def _build(extents, window, mode):
    nj = max((len(w) for w in window), default=0)
    n_mask = max(1, nj if mode != 'stream' else
                 sum(len(w) for w in window))
    mdt = BF16 if mode == 'mm' else F32
    nc = bacc.Bacc("TRN2", target_bir_lowering=False, debug=False,
                   num_devices=N_CORES)
    qT = nc.dram_tensor("qT", [D, TQ], BF16, kind="ExternalInput")
    kT = nc.dram_tensor("kT", [D, T], BF16, kind="ExternalInput")
    vT = nc.dram_tensor("vT", [D, T], BF16, kind="ExternalInput")
    w = nc.dram_tensor("w", [D, 3 * H], BF16, kind="ExternalInput")
    maskp = nc.dram_tensor("maskp", [n_mask, P, QC], mdt,
                           kind="ExternalInput")
    atri = nc.dram_tensor("atri", [P, P], BF16, kind="ExternalInput")
    out = nc.dram_tensor("out", [TQ, H], F32, kind="ExternalOutput")

    Exp = mybir.ActivationFunctionType.Exp
    kt_lim = max(extents)
    n_kv_chunks = (kt_lim * P + XCW - 1) // XCW
    KPC = XCW // P               # k-tiles per chunk
    jmap = {}
    for s in range(N_SLOTS):
        for j, t in enumerate(window[s]):
            jmap[(s, t)] = j
    stream_order = {}
    for t in range(kt_lim):
        for s in range(N_SLOTS):
            if t < extents[s] and (s, t) in jmap:
                stream_order[(s, t)] = len(stream_order)

    with tile.TileContext(nc) as tc:
        with (
            tc.tile_pool(name="const", bufs=1) as cpool,
            tc.tile_pool(name="qkh", bufs=1) as qkhpool,
            tc.tile_pool(name="vh", bufs=1) as vhpool,
            tc.tile_pool(name="oacc", bufs=1, space="PSUM") as opool,
        ):
            w_sb = cpool.tile([P, D_TILES, 3 * H], BF16)
            nc.sync.dma_start(
                out=w_sb[:], in_=w.ap().rearrange("(dt p) n -> p dt n", p=P))
            ident_b = cpool.tile([P, P], BF16)
            make_identity(nc, ident_b[:])
            ident_f = cpool.tile([H + 1, H + 1], F32)
            make_identity(nc, ident_f[:])
            atri_sb = None
            maskp_sb = None
            if mode == 'mm':
                atri_sb = cpool.tile([P, P], BF16)
                nc.sync.dma_start(out=atri_sb[:], in_=atri.ap())
            if mode != 'stream' and nj:
                maskp_sb = cpool.tile([P, nj, QC], mdt)
                nc.sync.dma_start(
                    out=maskp_sb[:],
                    in_=maskp.ap().rearrange("j p q -> p j q"))

            qhT = qkhpool.tile([H, TQ], BF16, tag="qhT")
            khT = qkhpool.tile([H, T], BF16, tag="khT")
            vh1 = vhpool.tile([P, KT, H + 1], BF16)

            oaccs = [opool.tile([H + 1, QC], F32, tag=f"oacc{s}",
                                name=f"oacc{s}")
                     for s in range(N_SLOTS)]

            with (
                tc.tile_pool(name="xs", bufs=2) as xpool,
                tc.tile_pool(name="ps", bufs=2, space="PSUM") as spool,
                tc.tile_pool(name="pt", bufs=6) as ppool,
                tc.tile_pool(name="vsb", bufs=2) as vsbpool,
                tc.tile_pool(name="mt", bufs=3) as mpool,
                tc.tile_pool(name="osb", bufs=2) as osbpool,
                tc.tile_pool(name="rec", bufs=2) as recpool,
                tc.tile_pool(name="ob", bufs=2) as obpool,
            ):
                def proj_x(src_sb, wlo, whi, base, dst_cb):
                    """project XCW cols starting at base; dst_cb(col, psum_ap)
                    consumes each [64, 1024] psum."""
                    for half in range(XCW // 1024):
                        pp = spool.tile([whi - wlo, 1024], F32, tag="ps")
                        for n in range(2):
                            for dt_ in range(D_TILES):
                                nc.tensor.matmul(
                                    pp[:, n * QC:(n + 1) * QC],
                                    lhsT=w_sb[:, dt_, wlo:whi],
                                    rhs=src_sb[:, dt_,
                                               half * 1024 + n * QC:
                                               half * 1024 + (n + 1) * QC],
                                    start=(dt_ == 0),
                                    stop=(dt_ == D_TILES - 1))
                        dst_cb(base + half * 1024, pp)

                def emit_scores(t):
                    """scores (+mask) + exp for k-tile t; returns (live, pts)."""
                    live = [s for s in range(N_SLOTS) if t < extents[s]]
                    pts = {}
                    for pi in range(2):
                        ss = [s for s in live if s // 2 == pi]
                        if not ss:
                            continue
                        wt = spool.tile([P, 2 * QC], F32, tag="ps")
                        for s in ss:
                            ho = (s % 2) * QC
                            mm_mask = mode == 'mm' and (s, t) in jmap
                            nc.tensor.matmul(
                                wt[:, ho:ho + QC],
                                lhsT=khT[:, t * P:(t + 1) * P],
                                rhs=qhT[:, s * QC:(s + 1) * QC],
                                start=True, stop=not mm_mask)
                            if mm_mask:
                                nc.tensor.matmul(
                                    wt[:, ho:ho + QC],
                                    lhsT=atri_sb[:],
                                    rhs=maskp_sb[:, jmap[(s, t)], :],
                                    start=False, stop=True)
                            elif (s, t) in jmap:
                                if mode == 'add':
                                    nc.vector.tensor_add(
                                        wt[:, ho:ho + QC], wt[:, ho:ho + QC],
                                        maskp_sb[:, jmap[(s, t)], :])
                                else:
                                    m = mpool.tile([P, QC], F32, tag="m")
                                    nc.sync.dma_start(
                                        out=m[:],
                                        in_=maskp[stream_order[(s, t)]])
                                    nc.vector.tensor_add(
                                        wt[:, ho:ho + QC], wt[:, ho:ho + QC],
                                        m[:])
                        lo = (min(ss) % 2) * QC
                        hi = (max(ss) % 2) * QC + QC
                        pt = ppool.tile([P, 2 * QC], BF16, tag="p")
                        nc.scalar.activation(
                            out=pt[:, lo:hi], in_=wt[:, lo:hi],
                            func=Exp, scale=0.125)
                        pts[pi] = pt
                    return live, pts

                def emit_pv(t, live, pts):
                    for s in live:
                        ho = (s % 2) * QC
                        nc.tensor.matmul(
                            oaccs[s][:],
                            lhsT=vh1[:, t, :],
                            rhs=pts[s // 2][:, ho:ho + QC],
                            start=(t == 0), stop=(t == extents[s] - 1))

                def epilogue(s):
                    osb = osbpool.tile([H + 1, QC], F32, tag="osb")
                    nc.vector.tensor_scalar_mul(osb[:], oaccs[s][:], 1.0)
                    for jj in range(QC // P):
                        ot = spool.tile([P, H + 1], F32, tag="ps")
                        nc.tensor.transpose(
                            ot[:], osb[:, jj * P:(jj + 1) * P],
                            ident_f[:])
                        rec = recpool.tile([P, 1], F32, tag="rec")
                        nc.vector.reciprocal(rec[:], ot[:, H:H + 1])
                        ob = obpool.tile([P, H], F32, tag="ob")
                        nc.vector.tensor_scalar_mul(ob[:], ot[:, 0:H], rec[:])
                        r0 = s * QC + jj * P
                        nc.sync.dma_start(out=out[r0:r0 + P, :], in_=ob[:])

                # ---- phase 0: project q -> qhT [64, TQ] ----
                for ch in range(TQ // XCW):
                    xt = xpool.tile([P, D_TILES, XCW], BF16, tag="x")
                    nc.sync.dma_start(
                        out=xt[:],
                        in_=qT[:, ch * XCW:(ch + 1) * XCW].rearrange(
                            "(dt p) t -> p dt t", p=P))
                    proj_x(xt, 0, H, ch * XCW,
                           lambda col, pp: nc.vector.tensor_scalar_mul(
                               qhT[:, col:col + 1024], pp[:], 1.0))

                # ---- phase 1: stream k/v chunks; project; attention ----
                prev = None          # (t, live, pts) awaiting PV
                for ch in range(n_kv_chunks):
                    ktx = xpool.tile([P, D_TILES, XCW], BF16, tag="x")
                    nc.sync.dma_start(
                        out=ktx[:],
                        in_=kT[:, ch * XCW:(ch + 1) * XCW].rearrange(
                            "(dt p) t -> p dt t", p=P))
                    proj_x(ktx, H, 2 * H, ch * XCW,
                           lambda col, pp: nc.vector.tensor_scalar_mul(
                               khT[:, col:col + 1024], pp[:], 1.0))

                    vtx = xpool.tile([P, D_TILES, XCW], BF16, tag="x")
                    nc.sync.dma_start(
                        out=vtx[:],
                        in_=vT[:, ch * XCW:(ch + 1) * XCW].rearrange(
                            "(dt p) t -> p dt t", p=P))
                    vsb = vsbpool.tile([H + 1, XCW], BF16, tag="vsb")
                    nc.vector.memset(vsb[H:H + 1, :], 1.0)
                    proj_x(vtx, 2 * H, 3 * H, ch * XCW,
                           lambda col, pp: nc.vector.tensor_scalar_mul(
                               vsb[0:H, col - ch * XCW:col - ch * XCW + 1024],
                               pp[:], 1.0))
                    for kt_ in range(KPC):
                        t_glob = ch * KPC + kt_
                        if t_glob >= kt_lim:
                            continue
                        tp = spool.tile([P, H + 1], BF16, tag="ps")
                        nc.tensor.transpose(
                            tp[:], vsb[:, kt_ * P:(kt_ + 1) * P],
                            ident_b[0:H + 1, 0:H + 1])
                        nc.vector.tensor_scalar_mul(
                            vh1[:, t_glob, :], tp[:], 1.0)

                    for t in range(ch * KPC, min((ch + 1) * KPC, kt_lim)):
                        cur = emit_scores(t)
                        if prev is not None:
                            pt_, live_, pts_ = prev
                            emit_pv(pt_, live_, pts_)
                            for s in live_:
                                if pt_ == extents[s] - 1:
                                    epilogue(s)
                        prev = (t, cur[0], cur[1])
                if prev is not None:
                    pt_, live_, pts_ = prev
                    emit_pv(pt_, live_, pts_)
                    for s in live_:
                        if pt_ == extents[s] - 1:
                            epilogue(s)

    nc.compile()
    return nc


def _get_nc(extents, window, mode):
    key = (extents, window, mode)
    if key not in _CACHE:
        _CACHE[key] = _build(extents, window, mode)
    return _CACHE[key]


def _pack_w(Wq, Wk, Wv):
    return np.concatenate(
        [np.asarray(Wq), np.asarray(Wk), np.asarray(Wv)],
        axis=1).astype(BF)


def _atri():
    i = np.arange(P)
    return (np.float32(NEG) * (i[:, None] <= i[None, :])).astype(BF)


def _make_in_maps(q, k, v, wcat, mask, extents, window, mode):
    nj = max((len(w) for w in window), default=0)
    kTb = [np.ascontiguousarray(k[b].T.astype(BF)) for b in range(B)]
    vTb = [np.ascontiguousarray(v[b].T.astype(BF)) for b in range(B)]
    qTb = [np.ascontiguousarray(q[b].T.astype(BF)) for b in range(B)]
    mm = mask.reshape(N_CHUNKS, QC, KT, P)
    atri = _atri()

    def add_tile(g, t):
        # [128 k, 512 q] additive f32 tile for (chunk g, k-tile t)
        return np.where(mm[g, :, t, :].T, np.float32(0.0), np.float32(NEG))

    def onehot_tile(g, t):
        # [128 i, 512 q] bf16: one-hot at i = first dropped k (suffix drop)
        drop = ~mm[g, :, t, :]                  # [qr, kc]
        any_drop = drop.any(axis=1)
        k0 = np.argmax(drop, axis=1)            # first dropped kc per qr
        b = np.zeros((P, QC), np.float32)
        b[k0[any_drop], np.nonzero(any_drop)[0]] = 1.0
        return b.astype(BF)

    in_maps = []
    for c in range(N_CORES):
        b, par = divmod(c, 2)
        chunks = [2 * s + par for s in range(N_SLOTS)]
        qT_core = np.ascontiguousarray(np.concatenate(
            [qTb[b][:, g * QC:(g + 1) * QC] for g in chunks], axis=1))
        if mode == 'stream':
            order = sorted(
                ((s, t) for s in range(N_SLOTS) for t in window[s]),
                key=lambda st: (st[1], st[0]))
            mp = np.stack([add_tile(chunks[s], t) for (s, t) in order]
                          ).astype(np.float32)
        elif nj:
            tiles = []
            for j in range(nj):
                s = next(s for s in range(N_SLOTS) if j < len(window[s]))
                g, t = chunks[s], window[s][j]
                tiles.append(onehot_tile(g, t) if mode == 'mm'
                             else add_tile(g, t).astype(np.float32))
            mp = np.stack(tiles)
        else:
            mp = np.zeros((1, P, QC), BF if mode == 'mm' else np.float32)
        in_maps.append({
            "qT": qT_core, "kT": kTb[b], "vT": vTb[b],
            "w": wcat, "maskp": mp, "atri": atri,
        })
    return in_maps


def _gather_out(results):
    outp = np.empty((B, T, H), np.float32)
    for c in range(N_CORES):
        b, par = divmod(c, 2)
        oc = results[c]["out"]
        for s in range(N_SLOTS):
            g = 2 * s + par
            outp[b, g * QC:(g + 1) * QC, :] = oc[s * QC:(s + 1) * QC, :]
    return outp


def kernel(q, k, v, Wq, Wk, Wv, attn_mask):
    global LAST_RESULT
    q = np.asarray(q, dtype=np.float32)
    k = np.asarray(k, dtype=np.float32)
    v = np.asarray(v, dtype=np.float32)
    mask = np.asarray(attn_mask).astype(bool)
    wcat = _pack_w(Wq, Wk, Wv)

    extents, window, mode = _mask_schedule(mask)
    nc = _get_nc(extents, window, mode)
    in_maps = _make_in_maps(q, k, v, wcat, mask, extents, window, mode)

    res = run_bass_kernel_spmd(
        nc, in_maps, core_ids=list(range(N_CORES)),
        trace=bool(os.environ.get("KBENCH_TRACE")))
    LAST_RESULT = res
    return _gather_out(res.results)


# revision 7
# speedup vs baseline: 1.1109x; 1.0463x over previous
"""Distributed Trainium2 Bass kernel for a single attention head.

Problem (hardcoded): q,k,v [4, 4096, 1024] f32, Wq/Wk/Wv [1024, 64] f32,
attn_mask [4096, 4096] bool (True = keep).  out[b] = softmax(mask(q Wq (k Wk)^T) / 8) (v Wv).

Sharding: 8 cores; core c -> batch c//2, and query chunks {2s + c%2 : s in 0..3}
(512 rows each, interleaved so the causal-mask work per program slot is uniform
across cores -- all cores execute one SPMD program).

v4: bf16 operands (host casts).  The PE p-state ramps to 2.4 GHz only under
continuous execution, so projection work for chunk ch+1 is emitted as a
generator and drained as FILLER between the attention matmuls of chunk ch --
the PE never idles waiting on exp/mask, so matmuls run at full clock.
Masks are resident additive f32 tiles applied by the vector engine (off the
PE); score PSUMs for slot pairs share a [128,1024] tile so one activation
instruction computes exp for two slots; PV runs one k-tile behind scores.
"""

import os
import sys

sys.path.insert(0, "/opt/trn_rl_repo")

import numpy as np
import ml_dtypes

import concourse.bass as bass
import concourse.mybir as mybir
import concourse.tile as tile
from concourse import bacc
from concourse.bass_utils import run_bass_kernel_spmd
from concourse.masks import make_identity

F32 = mybir.dt.float32
BF16 = mybir.dt.bfloat16
BF = ml_dtypes.bfloat16

N_CORES = 8
B, T, D, H = 4, 4096, 1024, 64
P = 128                      # partitions
QC = 512                     # query chunk width
N_CHUNKS = T // QC           # 8 global query chunks
N_SLOTS = N_CHUNKS // 2      # 4 chunks per core
KT = T // P                  # 32 k-tiles of 128 rows
D_TILES = D // P             # 8
XCW = 1024                   # x-chunk width for streaming projections
TQ = N_SLOTS * QC            # 2048 local query rows per core
KPC = XCW // P               # k-tiles per chunk
NEG = -1.0e30                # additive mask value for dropped positions

LAST_RESULT = None           # test harness reads exec_time_ns from here
_CACHE = {}


def _mask_schedule(mask):
    """Derive the compile-time schedule from the actual mask.

    Returns (extents, window, mode):
      extents[s]: #k-tiles processed for slot s (valid for both parities)
      window[s]:  tuple of k-tiles t < extents[s] needing masking
      mode: 'add' if the j-th window block content is slot-independent for
            each parity (resident additive tiles), else 'stream'.
    """
    m = mask.reshape(N_CHUNKS, QC, KT, P)
    blk_any = m.any(axis=(1, 3))   # [chunk, ktile]
    blk_all = m.all(axis=(1, 3))
    extents = []
    window = []
    for s in range(N_SLOTS):
        js = (2 * s, 2 * s + 1)
        ext = 1
        for j in js:
            nz = np.nonzero(blk_any[j])[0]
            if len(nz):
                ext = max(ext, int(nz[-1]) + 1)
        extents.append(ext)
        window.append(tuple(
            t for t in range(ext) if (~blk_all[js, t]).any()))
    nj = max((len(w) for w in window), default=0)
    consistent = nj <= 16
    if consistent:
        for par in range(2):
            for j in range(nj):
                blocks = [
                    m[2 * s + par, :, window[s][j], :]
                    for s in range(N_SLOTS) if j < len(window[s])
                ]
                if any(not np.array_equal(blocks[0], bb) for bb in blocks[1:]):
                    consistent = False
    return tuple(extents), tuple(window), 'add' if consistent else 'stream'


def _build(extents, window, mode):
    nj = max((len(w) for w in window), default=0)
    n_mask = max(1, nj if mode == 'add' else sum(len(w) for w in window))
    nc = bacc.Bacc("TRN2", target_bir_lowering=False, debug=False,
                   num_devices=N_CORES)
    qT = nc.dram_tensor("qT", [D, TQ], BF16, kind="ExternalInput")
    kT = nc.dram_tensor("kT", [D, T], BF16, kind="ExternalInput")
    vT = nc.dram_tensor("vT", [D, T], BF16, kind="ExternalInput")
    w = nc.dram_tensor("w", [D, 3 * H], BF16, kind="ExternalInput")
    maskp = nc.dram_tensor("maskp", [n_mask, P, QC], F32, kind="ExternalInput")
    out = nc.dram_tensor("out", [TQ, H], F32, kind="ExternalOutput")

    Exp = mybir.ActivationFunctionType.Exp
    kt_lim = max(extents)
    n_kv_chunks = (kt_lim * P + XCW - 1) // XCW
    jmap = {}
    for s in range(N_SLOTS):
        for j, t in enumerate(window[s]):
            jmap[(s, t)] = j
    stream_order = {}
    for t in range(kt_lim):
        for s in range(N_SLOTS):
            if t < extents[s] and (s, t) in jmap:
                stream_order[(s, t)] = len(stream_order)

    with tile.TileContext(nc) as tc:
        with (
            tc.tile_pool(name="const", bufs=1) as cpool,
            tc.tile_pool(name="qkh", bufs=1) as qkhpool,
            tc.tile_pool(name="vh", bufs=1) as vhpool,
            tc.tile_pool(name="oacc", bufs=1, space="PSUM") as opool,
        ):
            w_sb = cpool.tile([P, D_TILES, 3 * H], BF16)
            nc.sync.dma_start(
                out=w_sb[:], in_=w.ap().rearrange("(dt p) n -> p dt n", p=P))
            ident_b = cpool.tile([P, P], BF16)
            make_identity(nc, ident_b[:])
            ident_f = cpool.tile([H + 1, H + 1], F32)
            make_identity(nc, ident_f[:])
            maskp_sb = None
            if mode == 'add' and nj:
                maskp_sb = cpool.tile([P, nj, QC], F32)
                nc.sync.dma_start(
                    out=maskp_sb[:],
                    in_=maskp.ap().rearrange("j p q -> p j q"))

            qhT = qkhpool.tile([H, TQ], BF16, tag="qhT")
            khT = qkhpool.tile([H, T], BF16, tag="khT")
            vh1 = vhpool.tile([P, KT, H + 1], BF16)

            oaccs = [opool.tile([H + 1, QC], F32, tag=f"oacc{s}",
                                name=f"oacc{s}")
                     for s in range(N_SLOTS)]

            with (
                tc.tile_pool(name="xs", bufs=3) as xpool,
                tc.tile_pool(name="ps", bufs=2, space="PSUM") as spool,
                tc.tile_pool(name="pt", bufs=6) as ppool,
                tc.tile_pool(name="vsb", bufs=2) as vsbpool,
                tc.tile_pool(name="mt", bufs=3) as mpool,
                tc.tile_pool(name="osb", bufs=2) as osbpool,
                tc.tile_pool(name="rec", bufs=2) as recpool,
                tc.tile_pool(name="ob", bufs=2) as obpool,
            ):
                def proj_chain(pp, src_sb, wlo, whi):
                    """yield-friendly 16-matmul chain filling pp [64,1024]."""
                    for n in range(2):
                        for dt_ in range(D_TILES):
                            nc.tensor.matmul(
                                pp[:, n * QC:(n + 1) * QC],
                                lhsT=w_sb[:, dt_, wlo:whi],
                                rhs=src_sb[:, dt_, n * QC:(n + 1) * QC],
                                start=(dt_ == 0), stop=(dt_ == D_TILES - 1))
                            if dt_ % 4 == 3:
                                yield

                def q_proj(xt, base):
                    pp = spool.tile([H, XCW], F32, tag="ps")
                    for _ in proj_chain(pp, xt, 0, H):
                        pass
                    nc.vector.tensor_scalar_mul(
                        qhT[:, base:base + XCW], pp[:], 1.0)

                def kv_gen(ch):
                    """generator emitting chunk ch's k/v projection; yields
                    at ~2-4 matmul granularity so it can fill PE gaps."""
                    base = ch * XCW
                    ktx = xts[ch][0]
                    pp = spool.tile([H, XCW], F32, tag="ps")
                    yield from proj_chain(pp, ktx, H, 2 * H)
                    nc.vector.tensor_scalar_mul(
                        khT[:, base:base + XCW], pp[:], 1.0)
                    yield
                    vtx = xts[ch][1]
                    vp = spool.tile([H, XCW], F32, tag="ps")
                    yield from proj_chain(vp, vtx, 2 * H, 3 * H)
                    vsb = vsbpool.tile([H + 1, XCW], BF16, tag="vsb")
                    nc.vector.memset(vsb[H:H + 1, :], 1.0)
                    nc.vector.tensor_scalar_mul(vsb[0:H, :], vp[:], 1.0)
                    yield
                    for kt_ in range(KPC):
                        t_glob = ch * KPC + kt_
                        if t_glob >= kt_lim:
                            continue
                        tp = spool.tile([P, H + 1], BF16, tag="ps")
                        nc.tensor.transpose(
                            tp[:], vsb[:, kt_ * P:(kt_ + 1) * P],
                            ident_b[0:H + 1, 0:H + 1])
                        nc.vector.tensor_scalar_mul(
                            vh1[:, t_glob, :], tp[:], 1.0)
                        if kt_ % 2 == 1:
                            yield

                def dma_chunk(ch):
                    ktx = xpool.tile([P, D_TILES, XCW], BF16, tag="x")
                    nc.sync.dma_start(
                        out=ktx[:],
                        in_=kT[:, ch * XCW:(ch + 1) * XCW].rearrange(
                            "(dt p) t -> p dt t", p=P))
                    vtx = xpool.tile([P, D_TILES, XCW], BF16, tag="x")
                    nc.sync.dma_start(
                        out=vtx[:],
                        in_=vT[:, ch * XCW:(ch + 1) * XCW].rearrange(
                            "(dt p) t -> p dt t", p=P))
                    xts[ch] = (ktx, vtx)

                def emit_scores(t):
                    live = [s for s in range(N_SLOTS) if t < extents[s]]
                    pts = {}
                    for pi in range(2):
                        ss = [s for s in live if s // 2 == pi]
                        if not ss:
                            continue
                        wt = spool.tile([P, 2 * QC], F32, tag="ps")
                        for s in ss:
                            ho = (s % 2) * QC
                            nc.tensor.matmul(
                                wt[:, ho:ho + QC],
                                lhsT=khT[:, t * P:(t + 1) * P],
                                rhs=qhT[:, s * QC:(s + 1) * QC],
                                start=True, stop=True)
                            if (s, t) in jmap:
                                if mode == 'add':
                                    nc.vector.tensor_add(
                                        wt[:, ho:ho + QC], wt[:, ho:ho + QC],
                                        maskp_sb[:, jmap[(s, t)], :])
                                else:
                                    m = mpool.tile([P, QC], F32, tag="m")
                                    nc.sync.dma_start(
                                        out=m[:],
                                        in_=maskp[stream_order[(s, t)]])
                                    nc.vector.tensor_add(
                                        wt[:, ho:ho + QC], wt[:, ho:ho + QC],
                                        m[:])
                        lo = (min(ss) % 2) * QC
                        hi = (max(ss) % 2) * QC + QC
                        pt = ppool.tile([P, 2 * QC], BF16, tag="p")
                        nc.scalar.activation(
                            out=pt[:, lo:hi], in_=wt[:, lo:hi],
                            func=Exp, scale=0.125)
                        pts[pi] = pt
                    return live, pts

                def emit_pv(t, live, pts):
                    for s in live:
                        ho = (s % 2) * QC
                        nc.tensor.matmul(
                            oaccs[s][:],
                            lhsT=vh1[:, t, :],
                            rhs=pts[s // 2][:, ho:ho + QC],
                            start=(t == 0), stop=(t == extents[s] - 1))

                def epilogue(s):
                    osb = osbpool.tile([H + 1, QC], F32, tag="osb")
                    nc.vector.tensor_scalar_mul(osb[:], oaccs[s][:], 1.0)
                    for jj in range(QC // P):
                        ot = spool.tile([P, H + 1], F32, tag="ps")
                        nc.tensor.transpose(
                            ot[:], osb[:, jj * P:(jj + 1) * P], ident_f[:])
                        rec = recpool.tile([P, 1], F32, tag="rec")
                        nc.vector.reciprocal(rec[:], ot[:, H:H + 1])
                        ob = obpool.tile([P, H], F32, tag="ob")
                        nc.vector.tensor_scalar_mul(ob[:], ot[:, 0:H], rec[:])
                        r0 = s * QC + jj * P
                        nc.sync.dma_start(out=out[r0:r0 + P, :], in_=ob[:])

                xts = {}

                # q: load + project (PE ramps up during this phase)
                qxts = []
                for chq in range(TQ // XCW):
                    xt = xpool.tile([P, D_TILES, XCW], BF16, tag="x")
                    nc.sync.dma_start(
                        out=xt[:],
                        in_=qT[:, chq * XCW:(chq + 1) * XCW].rearrange(
                            "(dt p) t -> p dt t", p=P))
                    qxts.append(xt)
                dma_chunk(0)
                for chq, xt in enumerate(qxts):
                    q_proj(xt, chq * XCW)
                # chunk 0 projection runs serially (nothing to hide it behind)
                for _ in kv_gen(0):
                    pass

                prev = None          # (t, live, pts) awaiting PV
                gen = None           # projection generator being drained
                for ch in range(n_kv_chunks):
                    if ch + 1 < n_kv_chunks:
                        dma_chunk(ch + 1)
                        gen = kv_gen(ch + 1)
                    else:
                        gen = None
                    tiles = range(ch * KPC, min((ch + 1) * KPC, kt_lim))
                    for t in tiles:
                        cur = emit_scores(t)
                        if gen is not None:
                            for _ in range(3):
                                if next(gen, 'DONE') == 'DONE':
                                    gen = None
                                    break
                        if prev is not None:
                            pt_, live_, pts_ = prev
                            emit_pv(pt_, live_, pts_)
                            for s in live_:
                                if pt_ == extents[s] - 1:
                                    epilogue(s)
                        prev = (t, cur[0], cur[1])
                    if gen is not None:       # finish leftover projection
                        for _ in gen:
                            pass
                        gen = None
                if prev is not None:
                    pt_, live_, pts_ = prev
                    emit_pv(pt_, live_, pts_)
                    for s in live_:
                        if pt_ == extents[s] - 1:
                            epilogue(s)

    nc.compile()
    return nc


def _get_nc(extents, window, mode):
    key = (extents, window, mode)
    if key not in _CACHE:
        _CACHE[key] = _build(extents, window, mode)
    return _CACHE[key]


def _pack_w(Wq, Wk, Wv):
    return np.concatenate(
        [np.asarray(Wq), np.asarray(Wk), np.asarray(Wv)],
        axis=1).astype(BF)


def _make_in_maps(q, k, v, wcat, mask, extents, window, mode):
    nj = max((len(w) for w in window), default=0)
    kTb = [np.ascontiguousarray(k[b].T.astype(BF)) for b in range(B)]
    vTb = [np.ascontiguousarray(v[b].T.astype(BF)) for b in range(B)]
    qTb = [np.ascontiguousarray(q[b].T.astype(BF)) for b in range(B)]
    mm = mask.reshape(N_CHUNKS, QC, KT, P)

    def add_tile(g, t):
        # [128 k, 512 q] additive f32 tile for (chunk g, k-tile t)
        return np.where(mm[g, :, t, :].T, np.float32(0.0), np.float32(NEG))

    in_maps = []
    for c in range(N_CORES):
        b, par = divmod(c, 2)
        chunks = [2 * s + par for s in range(N_SLOTS)]
        qT_core = np.ascontiguousarray(np.concatenate(
            [qTb[b][:, g * QC:(g + 1) * QC] for g in chunks], axis=1))
        if mode == 'stream':
            order = sorted(
                ((s, t) for s in range(N_SLOTS) for t in window[s]),
                key=lambda st: (st[1], st[0]))
            mp = np.stack([add_tile(chunks[s], t) for (s, t) in order]
                          ).astype(np.float32)
        elif nj:
            tiles = []
            for j in range(nj):
                s = next(s for s in range(N_SLOTS) if j < len(window[s]))
                tiles.append(add_tile(chunks[s], window[s][j]))
            mp = np.stack(tiles).astype(np.float32)
        else:
            mp = np.zeros((1, P, QC), np.float32)
        in_maps.append({
            "qT": qT_core, "kT": kTb[b], "vT": vTb[b],
            "w": wcat, "maskp": mp,
        })
    return in_maps


def _gather_out(results):
    outp = np.empty((B, T, H), np.float32)
    for c in range(N_CORES):
        b, par = divmod(c, 2)
        oc = results[c]["out"]
        for s in range(N_SLOTS):
            g = 2 * s + par
            outp[b, g * QC:(g + 1) * QC, :] = oc[s * QC:(s + 1) * QC, :]
    return outp


def kernel(q, k, v, Wq, Wk, Wv, attn_mask):
    global LAST_RESULT
    q = np.asarray(q, dtype=np.float32)
    k = np.asarray(k, dtype=np.float32)
    v = np.asarray(v, dtype=np.float32)
    mask = np.asarray(attn_mask).astype(bool)
    wcat = _pack_w(Wq, Wk, Wv)

    extents, window, mode = _mask_schedule(mask)
    nc = _get_nc(extents, window, mode)
    in_maps = _make_in_maps(q, k, v, wcat, mask, extents, window, mode)

    res = run_bass_kernel_spmd(
        nc, in_maps, core_ids=list(range(N_CORES)),
        trace=bool(os.environ.get("KBENCH_TRACE")))
    LAST_RESULT = res
    return _gather_out(res.results)


# revision 11
# speedup vs baseline: 1.5185x; 1.3668x over previous
"""Distributed Trainium2 Bass kernel for a single attention head.

Problem (hardcoded): q,k,v [4, 4096, 1024] f32, Wq/Wk/Wv [1024, 64] f32,
attn_mask [4096, 4096] bool (True = keep).  out[b] = softmax(mask(q Wq (k Wk)^T) / 8) (v Wv).

Sharding: 8 cores; core c -> batch c//2, and query chunks {2s + c%2 : s in 0..3}
(512 rows each, interleaved so the causal-mask work per program slot is uniform
across cores -- all cores execute one SPMD program).

v5: bf16 operands (host casts).  The PE only reaches 2.4 GHz after ~3us of
gapless execution, so everything is arranged to keep its queue free of
cross-engine waits: (1) the causal mask is applied ON the PE (additive term
= Atri^T @ B accumulated into the score PSUM, with Atri[i,k]=NEG*(i<=k)
constant and B one-hot at the first dropped k per query -- no DVE add on
the score->exp chain); (2) projection work for chunk ch+1 is emitted as a
generator drained as filler between attention matmuls of chunk ch; (3) PV
runs one k-tile behind scores so exp latency is hidden; (4) score PSUMs for
slot pairs share a [128,1024] tile -> one exp instruction per pair; (5)
slot 3's first extents[0] k-tiles are deferred to the tail so slot 0 and
slot 3 share one PSUM bank, freeing a dedicated projection-PSUM bank
(avoids WAR stalls in the score-tile rotation).
"""

import os
import sys

sys.path.insert(0, "/opt/trn_rl_repo")

import numpy as np
import ml_dtypes

import concourse.bass as bass
import concourse.mybir as mybir
import concourse.tile as tile
from concourse import bacc
from concourse.bass_utils import run_bass_kernel_spmd
from concourse.masks import make_identity

F32 = mybir.dt.float32
BF16 = mybir.dt.bfloat16
BF = ml_dtypes.bfloat16

N_CORES = 8
B, T, D, H = 4, 4096, 1024, 64
P = 128                      # partitions
QC = 512                     # query chunk width
N_CHUNKS = T // QC           # 8 global query chunks
N_SLOTS = N_CHUNKS // 2      # 4 chunks per core
KT = T // P                  # 32 k-tiles of 128 rows
D_TILES = D // P             # 8
XCW = 1024                   # x-chunk width for streaming projections
TQ = N_SLOTS * QC            # 2048 local query rows per core
KPC = XCW // P               # k-tiles per chunk
NEG = -1.0e30                # additive mask value for dropped positions

LAST_RESULT = None           # test harness reads exec_time_ns from here
_CACHE = {}


def _mask_schedule(mask):
    """Derive the compile-time schedule from the actual mask.

    Returns (extents, window, mode):
      mode 'mm': window blocks are k-suffix drops, content slot-independent
                 per parity -> mask as a PE matmul (one-hot B tiles).
      mode 'add': consistent but not suffix -> resident additive DVE tiles.
      mode 'stream': general fallback, tiles streamed from HBM per (s,t).
    """
    m = mask.reshape(N_CHUNKS, QC, KT, P)
    blk_any = m.any(axis=(1, 3))   # [chunk, ktile]
    blk_all = m.all(axis=(1, 3))
    extents = []
    window = []
    for s in range(N_SLOTS):
        js = (2 * s, 2 * s + 1)
        ext = 1
        for j in js:
            nz = np.nonzero(blk_any[j])[0]
            if len(nz):
                ext = max(ext, int(nz[-1]) + 1)
        extents.append(ext)
        window.append(tuple(
            t for t in range(ext) if (~blk_all[js, t]).any()))
    nj = max((len(w) for w in window), default=0)
    consistent = nj <= 16
    suffix = True
    if consistent:
        for par in range(2):
            for j in range(nj):
                blocks = [
                    m[2 * s + par, :, window[s][j], :]
                    for s in range(N_SLOTS) if j < len(window[s])
                ]
                if any(not np.array_equal(blocks[0], bb) for bb in blocks[1:]):
                    consistent = False
                drop = ~blocks[0]            # [qr, kc]
                if not (drop[:, :-1] <= drop[:, 1:]).all():
                    suffix = False
    mode = ('mm' if consistent and suffix else
            'add' if consistent else 'stream')
    return tuple(extents), tuple(window), mode


def _build(extents, window, mode):
    nj = max((len(w) for w in window), default=0)
    n_mask = max(1, nj if mode != 'stream' else sum(len(w) for w in window))
    mdt = BF16 if mode == 'mm' else F32
    nc = bacc.Bacc("TRN2", target_bir_lowering=False, debug=False,
                   num_devices=N_CORES)
    qT = nc.dram_tensor("qT", [D, TQ], BF16, kind="ExternalInput")
    kT = nc.dram_tensor("kT", [D, T], BF16, kind="ExternalInput")
    vT = nc.dram_tensor("vT", [D, T], BF16, kind="ExternalInput")
    w = nc.dram_tensor("w", [D, 3 * H], BF16, kind="ExternalInput")
    maskp = nc.dram_tensor("maskp", [n_mask, P, QC], mdt,
                           kind="ExternalInput")
    atri = nc.dram_tensor("atri", [P, P], BF16, kind="ExternalInput")
    out = nc.dram_tensor("out", [TQ, H], F32, kind="ExternalOutput")

    Exp = mybir.ActivationFunctionType.Exp
    kt_lim = max(extents)
    n_kv_chunks = (kt_lim * P + XCW - 1) // XCW
    jmap = {}
    for s in range(N_SLOTS):
        for j, t in enumerate(window[s]):
            jmap[(s, t)] = j
    stream_order = {}
    for t in range(kt_lim):
        for s in range(N_SLOTS):
            if t < extents[s] and (s, t) in jmap:
                stream_order[(s, t)] = len(stream_order)

    # slot-3 deferral: share one PSUM bank between slot 0 and slot 3 by
    # processing slot 3's first extents[0] k-tiles after everything else
    # (PSUM accumulation is order-free).  Needs extents[3] > extents[0].
    DEF = N_SLOTS - 1
    E0 = extents[0]
    defer = extents[DEF] > E0
    # per-slot processed-tile order (for start/stop flags)
    order_of = {}
    for s in range(N_SLOTS):
        if s == DEF and defer:
            order_of[s] = list(range(E0, extents[s])) + list(range(E0))
        else:
            order_of[s] = list(range(extents[s]))
    first_t = {s: order_of[s][0] for s in range(N_SLOTS)}
    last_t = {s: order_of[s][-1] for s in range(N_SLOTS)}
    # schedule: (t, [slots]) groups in emission order
    sched = []
    for t in range(kt_lim):
        live = [s for s in range(N_SLOTS)
                if t < extents[s] and not (defer and s == DEF and t < E0)]
        if live:
            sched.append((t, live))
    if defer:
        for t in range(E0):
            sched.append((t, [DEF]))

    with tile.TileContext(nc) as tc:
        with (
            tc.tile_pool(name="const", bufs=1) as cpool,
            tc.tile_pool(name="qkh", bufs=1) as qkhpool,
            tc.tile_pool(name="vh", bufs=1) as vhpool,
            tc.tile_pool(name="oacc", bufs=1, space="PSUM") as opool,
        ):
            w_sb = cpool.tile([P, D_TILES, 3 * H], BF16)
            nc.sync.dma_start(
                out=w_sb[:], in_=w.ap().rearrange("(dt p) n -> p dt n", p=P))
            ident_b = cpool.tile([P, P], BF16)
            make_identity(nc, ident_b[:])
            ident_f = cpool.tile([H + 1, H + 1], F32)
            make_identity(nc, ident_f[:])
            atri_sb = None
            maskp_sb = None
            if mode == 'mm':
                atri_sb = cpool.tile([P, P], BF16)
                nc.sync.dma_start(out=atri_sb[:], in_=atri.ap())
            if mode != 'stream' and nj:
                maskp_sb = cpool.tile([P, nj, QC], mdt)
                nc.sync.dma_start(
                    out=maskp_sb[:],
                    in_=maskp.ap().rearrange("j p q -> p j q"))

            qhT = qkhpool.tile([H, TQ], BF16, tag="qhT")
            khT = qkhpool.tile([H, T], BF16, tag="khT")
            vh1 = vhpool.tile([P, KT, H + 1], BF16)

            # slot -> (tag, name); slot DEF shares slot 0's bank when deferred
            oaccs = {}
            def oacc_alloc(s):
                tag = "oaccA" if (s in (0, DEF) and defer) else f"oacc{s}"
                oaccs[s] = opool.tile([H + 1, QC], F32, tag=tag,
                                      name=f"oacc{s}")
            for s in range(N_SLOTS):
                if not (defer and s == DEF):
                    oacc_alloc(s)

            with (
                tc.tile_pool(name="xs", bufs=3) as xpool,
                tc.tile_pool(name="ps", bufs=2, space="PSUM") as spool,
                tc.tile_pool(name="pps", bufs=1, space="PSUM") as pppool,
                tc.tile_pool(name="pt", bufs=6) as ppool,
                tc.tile_pool(name="vsb", bufs=2) as vsbpool,
                tc.tile_pool(name="mt", bufs=3) as mpool,
                tc.tile_pool(name="osb", bufs=2) as osbpool,
                tc.tile_pool(name="rec", bufs=2) as recpool,
                tc.tile_pool(name="ob", bufs=2) as obpool,
            ):
                proj_pool = pppool if defer else spool

                def proj_chain(dst, dst_col, src_sb, n, wlo, whi):
                    """8-matmul chain projecting src cols [n*QC,(n+1)*QC);
                    DVE-copies the psum into dst[:, dst_col:dst_col+QC]."""
                    pp = proj_pool.tile([H, QC], F32, tag="pp")
                    for dt_ in range(D_TILES):
                        nc.tensor.matmul(
                            pp[:],
                            lhsT=w_sb[:, dt_, wlo:whi],
                            rhs=src_sb[:, dt_, n * QC:(n + 1) * QC],
                            start=(dt_ == 0), stop=(dt_ == D_TILES - 1))
                        if dt_ % 4 == 3:
                            yield
                    nc.vector.tensor_scalar_mul(
                        dst[0:H, dst_col:dst_col + QC], pp[:], 1.0)
                    yield

                def kv_gen(ch):
                    """chunk ch's k/v projection, yielded in filler units."""
                    base = ch * XCW
                    ktx, vtx = xts[ch]
                    for n in range(XCW // QC):
                        yield from proj_chain(khT, base + n * QC, ktx, n,
                                              H, 2 * H)
                    vsb = vsbpool.tile([H + 1, XCW], BF16, tag="vsb")
                    nc.vector.memset(vsb[H:H + 1, :], 1.0)
                    for n in range(XCW // QC):
                        yield from proj_chain(vsb, n * QC, vtx, n,
                                              2 * H, 3 * H)
                    for kt_ in range(KPC):
                        t_glob = ch * KPC + kt_
                        if t_glob >= kt_lim:
                            continue
                        tp = proj_pool.tile([P, H + 1], BF16, tag="pp")
                        nc.tensor.transpose(
                            tp[:], vsb[:, kt_ * P:(kt_ + 1) * P],
                            ident_b[0:H + 1, 0:H + 1])
                        nc.vector.tensor_scalar_mul(
                            vh1[:, t_glob, :], tp[:], 1.0)
                        if kt_ % 2 == 1:
                            yield

                def dma_chunk(ch):
                    ktx = xpool.tile([P, D_TILES, XCW], BF16, tag="x")
                    nc.sync.dma_start(
                        out=ktx[:],
                        in_=kT[:, ch * XCW:(ch + 1) * XCW].rearrange(
                            "(dt p) t -> p dt t", p=P))
                    vtx = xpool.tile([P, D_TILES, XCW], BF16, tag="x")
                    nc.sync.dma_start(
                        out=vtx[:],
                        in_=vT[:, ch * XCW:(ch + 1) * XCW].rearrange(
                            "(dt p) t -> p dt t", p=P))
                    xts[ch] = (ktx, vtx)

                def emit_scores(t, live):
                    pts = {}
                    for pi in range(2):
                        ss = [s for s in live if s // 2 == pi]
                        if not ss:
                            continue
                        wt = spool.tile([P, 2 * QC], F32, tag="ps")
                        for s in ss:
                            ho = (s % 2) * QC
                            mm_mask = mode == 'mm' and (s, t) in jmap
                            nc.tensor.matmul(
                                wt[:, ho:ho + QC],
                                lhsT=khT[:, t * P:(t + 1) * P],
                                rhs=qhT[:, s * QC:(s + 1) * QC],
                                start=True, stop=not mm_mask)
                            if mm_mask:
                                nc.tensor.matmul(
                                    wt[:, ho:ho + QC],
                                    lhsT=atri_sb[:],
                                    rhs=maskp_sb[:, jmap[(s, t)], :],
                                    start=False, stop=True)
                            elif (s, t) in jmap:
                                if mode == 'add':
                                    nc.vector.tensor_add(
                                        wt[:, ho:ho + QC], wt[:, ho:ho + QC],
                                        maskp_sb[:, jmap[(s, t)], :])
                                else:
                                    m = mpool.tile([P, QC], F32, tag="m")
                                    nc.sync.dma_start(
                                        out=m[:],
                                        in_=maskp[stream_order[(s, t)]])
                                    nc.vector.tensor_add(
                                        wt[:, ho:ho + QC], wt[:, ho:ho + QC],
                                        m[:])
                        lo = (min(ss) % 2) * QC
                        hi = (max(ss) % 2) * QC + QC
                        pt = ppool.tile([P, 2 * QC], BF16, tag="p")
                        nc.scalar.activation(
                            out=pt[:, lo:hi], in_=wt[:, lo:hi],
                            func=Exp, scale=0.125)
                        pts[pi] = pt
                    return pts

                def emit_pv(t, live, pts):
                    for s in live:
                        if s not in oaccs:
                            oacc_alloc(s)
                        ho = (s % 2) * QC
                        nc.tensor.matmul(
                            oaccs[s][:],
                            lhsT=vh1[:, t, :],
                            rhs=pts[s // 2][:, ho:ho + QC],
                            start=(t == first_t[s]), stop=(t == last_t[s]))

                def epilogue(s):
                    osb = osbpool.tile([H + 1, QC], F32, tag="osb")
                    nc.vector.tensor_scalar_mul(osb[:], oaccs[s][:], 1.0)
                    for jj in range(QC // P):
                        ot = proj_pool.tile([P, H + 1], F32, tag="pp")
                        nc.tensor.transpose(
                            ot[:], osb[:, jj * P:(jj + 1) * P], ident_f[:])
                        rec = recpool.tile([P, 1], F32, tag="rec")
                        nc.vector.reciprocal(rec[:], ot[:, H:H + 1])
                        ob = obpool.tile([P, H], F32, tag="ob")
                        nc.vector.tensor_scalar_mul(ob[:], ot[:, 0:H], rec[:])
                        r0 = s * QC + jj * P
                        nc.sync.dma_start(out=out[r0:r0 + P, :], in_=ob[:])

                xts = {}

                # q: load + project
                qxts = []
                for chq in range(TQ // XCW):
                    xt = xpool.tile([P, D_TILES, XCW], BF16, tag="x")
                    nc.sync.dma_start(
                        out=xt[:],
                        in_=qT[:, chq * XCW:(chq + 1) * XCW].rearrange(
                            "(dt p) t -> p dt t", p=P))
                    qxts.append(xt)
                dma_chunk(0)
                for chq, xt in enumerate(qxts):
                    for n in range(XCW // QC):
                        for _ in proj_chain(qhT, chq * XCW + n * QC, xt, n,
                                            0, H):
                            pass
                for _ in kv_gen(0):
                    pass

                # filler pacing: while emitting chunk c's attention tiles,
                # drain chunk c+1's projection generator between matmuls.
                prev = None          # (t, live, pts) awaiting PV
                gen = None
                gen_ch = 0           # chunk the current gen belongs to
                seen_ch = -1
                for (t, live) in sched:
                    ch = t // KPC
                    if ch > seen_ch:
                        seen_ch = ch
                        if gen is not None:   # leftover: must finish now
                            for _ in gen:
                                pass
                            gen = None
                        if ch + 1 < n_kv_chunks and ch + 1 > gen_ch:
                            dma_chunk(ch + 1)
                            gen = kv_gen(ch + 1)
                            gen_ch = ch + 1
                    cur = emit_scores(t, live)
                    if gen is not None:
                        for _ in range(3):
                            if next(gen, 'DONE') == 'DONE':
                                gen = None
                                break
                    if prev is not None:
                        pt_, live_, pts_ = prev
                        emit_pv(pt_, live_, pts_)
                        for s in live_:
                            if pt_ == last_t[s]:
                                epilogue(s)
                    prev = (t, live, cur)
                if gen is not None:
                    for _ in gen:
                        pass
                if prev is not None:
                    pt_, live_, pts_ = prev
                    emit_pv(pt_, live_, pts_)
                    for s in live_:
                        if pt_ == last_t[s]:
                            epilogue(s)

    nc.compile()
    return nc


def _get_nc(extents, window, mode):
    key = (extents, window, mode)
    if key not in _CACHE:
        _CACHE[key] = _build(extents, window, mode)
    return _CACHE[key]


def _pack_w(Wq, Wk, Wv):
    return np.concatenate(
        [np.asarray(Wq), np.asarray(Wk), np.asarray(Wv)],
        axis=1).astype(BF)


def _atri():
    i = np.arange(P)
    return (np.float32(NEG) * (i[:, None] <= i[None, :])).astype(BF)


def _make_in_maps(q, k, v, wcat, mask, extents, window, mode):
    nj = max((len(w) for w in window), default=0)
    kTb = [np.ascontiguousarray(k[b].T.astype(BF)) for b in range(B)]
    vTb = [np.ascontiguousarray(v[b].T.astype(BF)) for b in range(B)]
    qTb = [np.ascontiguousarray(q[b].T.astype(BF)) for b in range(B)]
    mm = mask.reshape(N_CHUNKS, QC, KT, P)
    atri = _atri()

    def add_tile(g, t):
        # [128 k, 512 q] additive f32 tile for (chunk g, k-tile t)
        return np.where(mm[g, :, t, :].T, np.float32(0.0), np.float32(NEG))

    def onehot_tile(g, t):
        # [128 i, 512 q] bf16: one-hot at i = first dropped k (suffix drop)
        drop = ~mm[g, :, t, :]                  # [qr, kc]
        any_drop = drop.any(axis=1)
        k0 = np.argmax(drop, axis=1)
        b = np.zeros((P, QC), np.float32)
        b[k0[any_drop], np.nonzero(any_drop)[0]] = 1.0
        return b.astype(BF)

    in_maps = []
    for c in range(N_CORES):
        b, par = divmod(c, 2)
        chunks = [2 * s + par for s in range(N_SLOTS)]
        qT_core = np.ascontiguousarray(np.concatenate(
            [qTb[b][:, g * QC:(g + 1) * QC] for g in chunks], axis=1))
        if mode == 'stream':
            order = sorted(
                ((s, t) for s in range(N_SLOTS) for t in window[s]),
                key=lambda st: (st[1], st[0]))
            mp = np.stack([add_tile(chunks[s], t) for (s, t) in order]
                          ).astype(np.float32)
        elif nj:
            tiles = []
            for j in range(nj):
                s = next(s for s in range(N_SLOTS) if j < len(window[s]))
                g, t = chunks[s], window[s][j]
                tiles.append(onehot_tile(g, t) if mode == 'mm'
                             else add_tile(g, t).astype(np.float32))
            mp = np.stack(tiles)
        else:
            mp = np.zeros((1, P, QC), BF if mode == 'mm' else np.float32)
        in_maps.append({
            "qT": qT_core, "kT": kTb[b], "vT": vTb[b],
            "w": wcat, "maskp": mp, "atri": atri,
        })
    return in_maps


def _gather_out(results):
    outp = np.empty((B, T, H), np.float32)
    for c in range(N_CORES):
        b, par = divmod(c, 2)
        oc = results[c]["out"]
        for s in range(N_SLOTS):
            g = 2 * s + par
            outp[b, g * QC:(g + 1) * QC, :] = oc[s * QC:(s + 1) * QC, :]
    return outp


def kernel(q, k, v, Wq, Wk, Wv, attn_mask):
    global LAST_RESULT
    q = np.asarray(q, dtype=np.float32)
    k = np.asarray(k, dtype=np.float32)
    v = np.asarray(v, dtype=np.float32)
    mask = np.asarray(attn_mask).astype(bool)
    wcat = _pack_w(Wq, Wk, Wv)

    extents, window, mode = _mask_schedule(mask)
    nc = _get_nc(extents, window, mode)
    in_maps = _make_in_maps(q, k, v, wcat, mask, extents, window, mode)

    res = run_bass_kernel_spmd(
        nc, in_maps, core_ids=list(range(N_CORES)),
        trace=bool(os.environ.get("KBENCH_TRACE")))
    LAST_RESULT = res
    return _gather_out(res.results)
